# revision 1
# baseline (speedup 1.0000x reference)
"""ARAP loss (nn_ARAPLoss) on 8 Trainium2 NeuronCores — self-contained kernel.

Sharding: points (dim 0 of all [N,K] buffers) split contiguously across 8
cores (250,000 each, padded to 250,880 = 128*1960). The per-edge neighbor
streams are materialized host-side from the full point cloud; all per-edge
math runs on-device, fully data-parallel; per-partition partial sums are
reduced to a [128, 2] output per core and combined to the scalar on host.

Per-core inputs (P = 128 partitions, rows_pp points per partition):
  gp   [P, rows_pp*K*3] f32   gathered p_j, edge-major (e, d)
  gr   [P, rows_pp*3*K] f32   gathered r_j = p_j - q_j, k-major (c, d, k)
  dist [P, rows_pp*K]   f32
  w    [P, rows_pp*K]   f32
  pc   [P, rows_pp*3]   f32
  q    [P, rows_pp*3]   f32
Output: out [P, 2] f32 — col 0 = sum |(||p_i-p_j||^2 - d)*w|,
                         col 1 = sum |(p_i - q_i) - mean_k r_j|
Padding rows use point 0's data with w = 0 so both terms contribute ~0.
"""

import sys
import types

import numpy as np
import ml_dtypes

try:
    import antenv.axon_hooks  # noqa: F401
except ImportError:
    mod = types.ModuleType("antenv.axon_hooks")
    mod._hook = None

    def _set(hook):
        mod._hook = hook

    def _get():
        return mod._hook

    mod.set_axon_ntff_profile_hook = _set
    mod.get_axon_ntff_profile_hook = _get
    sys.modules["antenv.axon_hooks"] = mod
    try:
        from trn_agent_boot.trn_boot import _ntff_profile_via_ctypes

        _set(_ntff_profile_via_ctypes("/opt/axon/libaxon_pjrt.so"))
    except Exception:
        pass

import concourse.bacc as bacc
import concourse.mybir as mybir
import concourse.tile as tile
from concourse.bass_utils import run_bass_kernel_spmd

F32 = mybir.dt.float32
BF16 = mybir.dt.bfloat16
P = 128
N = 2_000_000
K = 10
N_CORES = 8
ROWS_PP = 1960
CHUNK = 140
LDA_WEIGHT = 1.0

LAST_RUN_INFO = {}
_NC_CACHE = {}


def _build_kernel(rows_pp, chunk_pts):
    n_chunks = rows_pp // chunk_pts
    C = chunk_pts
    E = C * K

    nc = bacc.Bacc(None, target_bir_lowering=False)

    gp_d = nc.dram_tensor("gp", [P, rows_pp * K * 3], BF16, kind="ExternalInput")
    gr_d = nc.dram_tensor("gr", [P, rows_pp * 3 * K], BF16, kind="ExternalInput")
    dist_d = nc.dram_tensor("dist", [P, rows_pp * K], BF16, kind="ExternalInput")
    w_d = nc.dram_tensor("w", [P, rows_pp * K], F32, kind="ExternalInput")
    pc_d = nc.dram_tensor("pc", [P, rows_pp * 3], F32, kind="ExternalInput")
    pq_d = nc.dram_tensor("pq", [P, rows_pp * 3], F32, kind="ExternalInput")
    out_d = nc.dram_tensor("out", [P, 2], F32, kind="ExternalOutput")

    with tile.TileContext(nc) as tc:
        with (
            tc.tile_pool(name="accp", bufs=1) as accp,
            tc.tile_pool(name="sbuf", bufs=3) as pool,
        ):
            acc = accp.tile([P, 2], F32)
            nc.vector.memset(acc[:], 0.0)

            for ci in range(n_chunks):
                e0 = ci * E
                p0 = ci * C * 3

                gp = pool.tile([P, E * 3], BF16)
                nc.sync.dma_start(out=gp[:], in_=gp_d[:, e0 * 3 : (e0 + E) * 3])
                gr = pool.tile([P, C * 3 * K], BF16)
                nc.sync.dma_start(out=gr[:], in_=gr_d[:, e0 * 3 : (e0 + E) * 3])
                dist_t = pool.tile([P, E], BF16)
                nc.sync.dma_start(out=dist_t[:], in_=dist_d[:, e0 : e0 + E])
                w_t = pool.tile([P, E + 16], F32)
                nc.sync.dma_start(out=w_t[:, :E], in_=w_d[:, e0 : e0 + E])
                pc_t = pool.tile([P, C * 3], F32)
                nc.sync.dma_start(out=pc_t[:], in_=pc_d[:, p0 : p0 + C * 3])
                pq = pool.tile([P, C * 3], F32)
                nc.sync.dma_start(out=pq[:], in_=pq_d[:, p0 : p0 + C * 3])

                # term 1 (planar layout: gp/pc arrive as x/y/z planes per chunk)
                diff = pool.tile([P, E * 3], BF16)
                gp_v = gp[:].rearrange("p (d c k) -> p d c k", d=3, k=K)
                pc_b = (
                    pc_t[:]
                    .rearrange("p (d c) -> p d c", d=3)
                    .unsqueeze(3)
                    .broadcast_to([P, 3, C, K])
                )
                diff_v = diff[:].rearrange("p (d c k) -> p d c k", d=3, k=K)
                nc.gpsimd.tensor_sub(diff_v, gp_v, pc_b)

                # square in place on the scalar engine
                nc.scalar.activation(
                    diff[:], diff[:], mybir.ActivationFunctionType.Square
                )

                # d2 = sq_x + sq_y + sq_z: plane inputs are fully contiguous
                sq_v = diff[:].rearrange("p (d e) -> p d e", d=3)
                d2 = pool.tile([P, E + 32], BF16)
                nc.vector.tensor_add(d2[:, :E], sq_v[:, 0, :], sq_v[:, 1, :])
                nc.vector.tensor_add(d2[:, :E], d2[:, :E], sq_v[:, 2, :])

                # u = d2 - dist on GpSimd (idle engine), |u| in place on ACT
                u = pool.tile([P, E + 48], F32)
                nc.gpsimd.tensor_sub(u[:, :E], d2[:, :E], dist_t[:])
                nc.scalar.activation(u[:, :E], u[:, :E], mybir.ActivationFunctionType.Abs)

                # sum_e |u|*w in one DVE pass (accumulator output)
                tmp1 = pool.tile([P, 1], F32)
                nc.vector.scalar_tensor_tensor(
                    out=diff[:, :E],  # dead bf16 tile, reused as scratch
                    in0=u[:, :E],
                    scalar=1.0,
                    in1=w_t[:, :E],
                    op0=mybir.AluOpType.mult,
                    op1=mybir.AluOpType.mult,
                    accum_out=tmp1[:],
                )
                nc.vector.tensor_add(acc[:, 0:1], acc[:, 0:1], tmp1[:])

                # LDA: gr is k-major (c, d, k) -> contiguous k-reduce
                s3 = pool.tile([P, C * 3], F32)
                nc.vector.tensor_reduce(
                    out=s3[:],
                    in_=gr[:].rearrange("p (cd k) -> p cd k", k=K),
                    axis=mybir.AxisListType.X,
                    op=mybir.AluOpType.add,
                )
                l = pool.tile([P, C * 3], F32)
                nc.vector.scalar_tensor_tensor(
                    out=l[:],
                    in0=s3[:],
                    scalar=-1.0 / K,
                    in1=pq[:],
                    op0=mybir.AluOpType.mult,
                    op1=mybir.AluOpType.add,
                )
                tmp2 = pool.tile([P, 1], F32)
                nc.vector.tensor_reduce(
                    out=tmp2[:],
                    in_=l[:],
                    axis=mybir.AxisListType.X,
                    op=mybir.AluOpType.add,
                    apply_absolute_value=True,
                )
                nc.vector.tensor_add(acc[:, 1:2], acc[:, 1:2], tmp2[:])

            nc.sync.dma_start(out=out_d[:], in_=acc[:])

    nc.compile()
    return nc


def _get_nc():
    key = (ROWS_PP, CHUNK)
    if key not in _NC_CACHE:
        _NC_CACHE[key] = _build_kernel(ROWS_PP, CHUNK)
    return _NC_CACHE[key]


def _shard_inputs(pc_tr, init_pos, idx_any, dists, weights):
    CH = CHUNK
    R = P * ROWS_PP
    base = N // N_CORES

    pc = np.ascontiguousarray(np.asarray(pc_tr, dtype=np.float32))
    q = np.ascontiguousarray(np.asarray(init_pos, dtype=np.float32))
    idx = np.asarray(idx_any, dtype=np.int64)
    dist = np.asarray(dists, dtype=np.float32)
    w = np.asarray(weights, dtype=np.float32)

    r_tab = pc - q

    in_maps = []
    for c in range(N_CORES):
        sl = slice(c * base, (c + 1) * base)
        idx_c = idx[sl].ravel()
        gp_e = np.empty((R, K, 3), np.float32)
        np.take(pc, idx_c, axis=0, out=gp_e[:base].reshape(-1, 3))
        gp_e[base:] = pc[0]
        # planar per (partition, chunk): [P, n_chunks, 3, C*K]
        nch = ROWS_PP // CH
        gp_s = np.ascontiguousarray(
            gp_e.reshape(P, nch, CH * K, 3).transpose(0, 1, 3, 2)
        )
        gr_s = np.empty((R, 3, K), np.float32)
        gr_s[:base] = r_tab[idx_c].reshape(base, K, 3).transpose(0, 2, 1)
        gr_s[base:] = r_tab[0][:, None]
        dist_s = np.zeros((R, K), np.float32)
        dist_s[:base] = dist[sl]
        w_s = np.zeros((R, K), np.float32)
        w_s[:base] = w[sl]
        pc_e = np.empty((R, 3), np.float32)
        pc_e[:base] = pc[sl]
        pc_e[base:] = pc[0]
        pc_s = np.ascontiguousarray(
            pc_e.reshape(P, nch, CH, 3).transpose(0, 1, 3, 2)
        )
        pq_s = np.empty((R, 3), np.float32)
        pq_s[:base] = pc[sl] - q[sl]
        pq_s[base:] = pc[0] - q[0]
        in_maps.append(
            {
                "gp": gp_s.reshape(P, ROWS_PP * K * 3).astype(ml_dtypes.bfloat16),
                "gr": gr_s.reshape(P, ROWS_PP * 3 * K).astype(ml_dtypes.bfloat16),
                "dist": dist_s.reshape(P, ROWS_PP * K).astype(ml_dtypes.bfloat16),
                "w": w_s.reshape(P, ROWS_PP * K),
                "pc": pc_s.reshape(P, ROWS_PP * 3),
                "pq": pq_s.reshape(P, ROWS_PP * 3),
            }
        )
    return in_maps


def kernel(pc_transformed, nn_init_positions, nn_indices, nn_distances, neighbor_weights):
    nc = _get_nc()
    in_maps = _shard_inputs(
        pc_transformed, nn_init_positions, nn_indices, nn_distances, neighbor_weights
    )
    try:
        res = run_bass_kernel_spmd(
            nc, in_maps, core_ids=list(range(N_CORES)), trace=True
        )
    except Exception:
        res = run_bass_kernel_spmd(
            nc, in_maps, core_ids=list(range(N_CORES)), trace=False
        )
    LAST_RUN_INFO["exec_time_ns"] = res.exec_time_ns
    LAST_RUN_INFO["mean_exec_time_ns"] = res.mean_exec_time_ns

    t1 = sum(
        float(res.results[i]["out"][:, 0].astype(np.float64).sum())
        for i in range(N_CORES)
    )
    t2 = sum(
        float(res.results[i]["out"][:, 1].astype(np.float64).sum())
        for i in range(N_CORES)
    )
    loss = t1 / (N * K) + LDA_WEIGHT * t2 / (N * 3)
    return np.float32(loss)



# revision 3
# speedup vs baseline: 1.0568x; 1.0568x over previous
"""ARAP loss (nn_ARAPLoss) on 8 Trainium2 NeuronCores — self-contained kernel.

v2: fp8 input streams, DMA-accumulated k-reduce, ACT-engine abs-accumulation.

Sharding: points (dim 0 of all [N,K] buffers) split contiguously across 8
cores (250,000 each, padded to 250,880 = 128*1960). The per-edge neighbor
streams are materialized host-side from the full point cloud; all per-edge
math runs on-device, fully data-parallel; per-partition partial sums land in
a [128, 9] accumulator per core and are combined to the scalar on host.

Per-core inputs (P = 128 partitions, rows = 1960 points per partition):
  pk  [P, nch*CB] fp8  packed per chunk: [gp 3E | dist E | w E | pc 3C]
                       gp planar (d, c, k); pc planar (d, c)
  grk [P, K*rows*3] fp8  gathered r_j = p_j - q_j, k-OUTERMOST: K slices
                         of [rows, 3] (c, d) — DMA-accumulated into s3
  pqk [P, rows*3]  bf16  K*(p_i - q_i), (c, d) order
Output: out [P, 9] f32 — cols 0..6 = per-chunk sum |(||p_i-p_j||^2 - d)*w|,
                         cols 7..8 = halves of sum |K*(p_i-q_i) - sum_k r_j|
Padding rows use point 0's data with w = 0 so both terms contribute ~0.
"""

import sys
import types

import numpy as np
import ml_dtypes

try:
    import antenv.axon_hooks  # noqa: F401
except ImportError:
    mod = types.ModuleType("antenv.axon_hooks")
    mod._hook = None

    def _set(hook):
        mod._hook = hook

    def _get():
        return mod._hook

    mod.set_axon_ntff_profile_hook = _set
    mod.get_axon_ntff_profile_hook = _get
    sys.modules["antenv.axon_hooks"] = mod
    try:
        from trn_agent_boot.trn_boot import _ntff_profile_via_ctypes

        _set(_ntff_profile_via_ctypes("/opt/axon/libaxon_pjrt.so"))
    except Exception:
        pass

import concourse.bacc as bacc
import concourse.mybir as mybir
import concourse.tile as tile
from concourse.bass_utils import run_bass_kernel_spmd

F32 = mybir.dt.float32
BF16 = mybir.dt.bfloat16
FP8 = mybir.dt.float8e4
P = 128
N = 2_000_000
K = 10
N_CORES = 8
ROWS = 1960
CHUNK = 280
LDA_WEIGHT = 1.0

NCH = ROWS // CHUNK
E = CHUNK * K
C3 = CHUNK * 3
CB = 3 * E + E + E + C3  # packed bytes (fp8 elems) per chunk per partition
R3 = ROWS * 3
RUN = R3 // 3          # 1960, <= 2048 CCE accum descriptor limit
SSTR = 2048            # padded run stride for the accum-DMA target
GW = 3 * SSTR          # padded k-slice width

LAST_RUN_INFO = {}
_NC_CACHE = {}


def _build_kernel():
    nc = bacc.Bacc(None, target_bir_lowering=False)

    pk_d = nc.dram_tensor("pk", [P, NCH * CB], FP8, kind="ExternalInput")
    grk_d = nc.dram_tensor("grk", [P, K * GW], FP8, kind="ExternalInput")
    pqk_d = nc.dram_tensor("pqk", [P, GW], BF16, kind="ExternalInput")
    out_d = nc.dram_tensor("out", [P, 10], F32, kind="ExternalOutput")

    add = mybir.AluOpType.add

    with tile.TileContext(nc) as tc:
        with (
            tc.tile_pool(name="statics", bufs=1) as statics,
            tc.tile_pool(name="sbuf", bufs=3) as pool,
        ):
            acc = statics.tile([P, 10], F32)
            s3 = statics.tile([P, GW], F32)
            pqk = statics.tile([P, GW], BF16)
            lsub = statics.tile([P, GW], BF16)

            nc.sync.dma_start(out=pqk[:], in_=pqk_d[:])

            # k-reduce of gathered r via DMA accumulate: first slice is a
            # plain cast-copy (initializes s3), the rest accumulate. Runs are
            # padded to a 2048 stride so each descriptor stays <= 2048 f32
            # elements (the CCE accumulate limit) and cannot coalesce.
            s3v = s3[:].rearrange("p (s f) -> p s f", s=3)[:, :, :RUN]
            for j in range(K):
                iv = grk_d[:, j * GW : (j + 1) * GW].rearrange(
                    "p (s f) -> p s f", s=3
                )[:, :, :RUN]
                nc.gpsimd.dma_start(
                    out=s3v,
                    in_=iv,
                    accum_op=add if j else mybir.AluOpType.bypass,
                )

            for ci in range(NCH):
                o = ci * CB
                pk = pool.tile([P, CB], FP8)
                nc.sync.dma_start(out=pk[:], in_=pk_d[:, o : o + CB])
                diff = pool.tile([P, 3 * E], BF16)

                gp_v = pk[:, : 3 * E].rearrange("p (d c k) -> p d c k", d=3, k=K)
                dist_v = pk[:, 3 * E : 4 * E]
                w_v = pk[:, 4 * E : 5 * E]
                pc_b = (
                    pk[:, 5 * E : 5 * E + C3]
                    .rearrange("p (d c) -> p d c", d=3)
                    .unsqueeze(3)
                    .broadcast_to([P, 3, CHUNK, K])
                )
                diff_v = diff[:].rearrange("p (d c k) -> p d c k", d=3, k=K)

                # diff = p_j - p_i (DVE), then squares (ACT, in place)
                nc.vector.tensor_sub(diff_v, gp_v, pc_b)
                nc.scalar.activation(
                    diff[:], diff[:], mybir.ActivationFunctionType.Square
                )

                # u = sq_x - dist + sq_y + sq_z, in place in the x-plane
                nc.vector.tensor_sub(diff[:, :E], diff[:, :E], dist_v)
                nc.vector.tensor_add(diff[:, :E], diff[:, :E], diff[:, E : 2 * E])
                nc.vector.tensor_add(diff[:, :E], diff[:, :E], diff[:, 2 * E : 3 * E])

                # t = u * w (GpSimd) into the y-plane; |t| summed on ACT
                nc.gpsimd.tensor_mul(diff[:, E : 2 * E], diff[:, :E], w_v)
                nc.scalar.activation(
                    diff[:, 2 * E : 3 * E],
                    diff[:, E : 2 * E],
                    mybir.ActivationFunctionType.Abs,
                    accum_out=acc[:, ci : ci + 1],
                )

            # LDA tail: l = s3 - K*(p_i - q_i); sum |l| per padded run
            for h in range(3):
                sl = slice(h * SSTR, h * SSTR + RUN)
                nc.vector.tensor_sub(lsub[:, sl], s3[:, sl], pqk[:, sl])
                nc.scalar.activation(
                    lsub[:, sl],
                    lsub[:, sl],
                    mybir.ActivationFunctionType.Abs,
                    accum_out=acc[:, 7 + h : 8 + h],
                )

            nc.sync.dma_start(out=out_d[:], in_=acc[:])

    nc.compile()
    return nc


def _get_nc():
    key = (ROWS, CHUNK)
    if key not in _NC_CACHE:
        _NC_CACHE[key] = _build_kernel()
    return _NC_CACHE[key]


def _shard_inputs(pc_tr, init_pos, idx_any, dists, weights):
    R = P * ROWS
    base = N // N_CORES
    f8 = ml_dtypes.float8_e4m3

    pc = np.ascontiguousarray(np.asarray(pc_tr, dtype=np.float32))
    q = np.ascontiguousarray(np.asarray(init_pos, dtype=np.float32))
    idx = np.asarray(idx_any, dtype=np.int64)
    dist = np.asarray(dists, dtype=np.float32)
    w = np.asarray(weights, dtype=np.float32)

    r_tab = pc - q

    in_maps = []
    for c in range(N_CORES):
        sl = slice(c * base, (c + 1) * base)
        idx_c = idx[sl].ravel()

        # gathered neighbor positions, planar (d, c, k) per chunk
        gp_e = np.empty((R, K, 3), np.float32)
        np.take(pc, idx_c, axis=0, out=gp_e[:base].reshape(-1, 3))
        gp_e[base:] = pc[0]
        gp_s = gp_e.reshape(P, NCH, CHUNK * K, 3).transpose(0, 1, 3, 2)

        dist_s = np.zeros((R, K), np.float32)
        dist_s[:base] = dist[sl]
        w_s = np.zeros((R, K), np.float32)
        w_s[:base] = w[sl]

        pc_e = np.empty((R, 3), np.float32)
        pc_e[:base] = pc[sl]
        pc_e[base:] = pc[0]
        pc_s = pc_e.reshape(P, NCH, CHUNK, 3).transpose(0, 1, 3, 2)

        # packed per-chunk stream: [gp 3E | dist E | w E | pc 3C]
        pk = np.empty((P, NCH, CB), f8)
        pk[:, :, : 3 * E] = gp_s.reshape(P, NCH, 3 * E).astype(f8)
        pk[:, :, 3 * E : 4 * E] = (
            dist_s.reshape(P, NCH, E).astype(f8)
        )
        pk[:, :, 4 * E : 5 * E] = w_s.reshape(P, NCH, E).astype(f8)
        pk[:, :, 5 * E :] = pc_s.reshape(P, NCH, C3).astype(f8)

        # gathered r, k-outermost [K, rows, 3] per partition
        gr_e = np.empty((R, K, 3), np.float32)
        np.take(r_tab, idx_c, axis=0, out=gr_e[:base].reshape(-1, 3))
        gr_e[base:] = r_tab[0]
        gr_f = gr_e.reshape(P, ROWS, K, 3).transpose(0, 2, 1, 3).reshape(P, K, R3)
        grk = np.zeros((P, K, 3, SSTR), np.float32)
        grk[:, :, :, :RUN] = gr_f.reshape(P, K, 3, RUN)

        pq_e = np.empty((R, 3), np.float32)
        pq_e[:base] = pc[sl] - q[sl]
        pq_e[base:] = r_tab[0]
        pq_f = (K * pq_e).reshape(P, 3, RUN)
        pqk = np.zeros((P, 3, SSTR), np.float32)
        pqk[:, :, :RUN] = pq_f

        in_maps.append(
            {
                "pk": pk.reshape(P, NCH * CB),
                "grk": grk.reshape(P, K * GW).astype(f8),
                "pqk": pqk.reshape(P, GW).astype(ml_dtypes.bfloat16),
            }
        )
    return in_maps


def kernel(pc_transformed, nn_init_positions, nn_indices, nn_distances, neighbor_weights):
    nc = _get_nc()
    in_maps = _shard_inputs(
        pc_transformed, nn_init_positions, nn_indices, nn_distances, neighbor_weights
    )
    try:
        res = run_bass_kernel_spmd(
            nc, in_maps, core_ids=list(range(N_CORES)), trace=True
        )
    except Exception:
        res = run_bass_kernel_spmd(
            nc, in_maps, core_ids=list(range(N_CORES)), trace=False
        )
    LAST_RUN_INFO["exec_time_ns"] = res.exec_time_ns
    LAST_RUN_INFO["mean_exec_time_ns"] = res.mean_exec_time_ns

    t1 = sum(
        float(res.results[i]["out"][:, :7].astype(np.float64).sum())
        for i in range(N_CORES)
    )
    t2 = sum(
        float(res.results[i]["out"][:, 7:10].astype(np.float64).sum())
        for i in range(N_CORES)
    )
    loss = t1 / (N * K) + LDA_WEIGHT * (t2 / K) / (N * 3)
    return np.float32(loss)


# revision 8
# speedup vs baseline: 1.6721x; 1.5823x over previous
"""ARAP loss (nn_ARAPLoss) on 8 Trainium2 NeuronCores — self-contained kernel.

v3: k-major (d,k,c) layout so the p_i broadcast keeps DVE in 2x mode,
TensorE identity-matmul PSUM accumulation for the neighbor k-reduce,
ACT-engine abs-accumulation, software-pipelined emission (depth 2).

Sharding: points (dim 0 of all [N,K] buffers) split contiguously across 8
cores (250,000 each, padded to 250,880 = 128*1960). The neighbor gathers are
materialized host-side from the full point cloud; all per-edge math runs
on-device, fully data-parallel; per-partition partial sums land in a
[128, 21] accumulator per core and are combined to the scalar on host.

Per-core inputs (P = 128 partitions, C = 280 points/partition/chunk, 7 chunks):
  pkb [P, nch*CBB] bf16 packed per chunk: [gp 3KC (d,k,c) | dist KC (k,c) |
                        pc 3C (d,c) | pqk 2x420 (h,c,d) = K*(p_i - q_i)]
  pkf [P, nch*CBF] fp8  packed per chunk: [gr (k,h,c,d) | w KC (k,c)]
  ident [P, 128] fp8    identity matrix for TensorE copy-accumulate
Output: out [P, 21] f32 — cols 0..6 = per-chunk sum |(||p_i-p_j||^2-d)*w|,
                          cols 7..20 = per (chunk, half) LDA partials
Padding rows use point 0's data with w = 0 so both terms contribute ~0.
"""

import sys
import types

import numpy as np
import ml_dtypes

try:
    import antenv.axon_hooks  # noqa: F401
except ImportError:
    mod = types.ModuleType("antenv.axon_hooks")
    mod._hook = None

    def _set(hook):
        mod._hook = hook

    def _get():
        return mod._hook

    mod.set_axon_ntff_profile_hook = _set
    mod.get_axon_ntff_profile_hook = _get
    sys.modules["antenv.axon_hooks"] = mod
    try:
        from trn_agent_boot.trn_boot import _ntff_profile_via_ctypes

        _set(_ntff_profile_via_ctypes("/opt/axon/libaxon_pjrt.so"))
    except Exception:
        pass

import concourse.bacc as bacc
import concourse.mybir as mybir
import concourse.tile as tile
from concourse.bass_utils import run_bass_kernel_spmd

F32 = mybir.dt.float32
BF16 = mybir.dt.bfloat16
FP8 = mybir.dt.float8e4
P = 128
N = 2_000_000
K = 10
N_CORES = 8
ROWS = 1960
CHUNK = 280
LDA_WEIGHT = 1.0

NCH = ROWS // CHUNK
E = CHUNK * K          # 2800 edges per partition per chunk
C3 = CHUNK * 3
HC = CHUNK // 2        # half-chunk points (PSUM bank limit: 420 f32 cols)
H3 = HC * 3            # 420
CBB = 3 * E + E + C3 + C3   # bf16 elems per chunk: gp, dist, pc, pqk
CBF = 3 * E + E             # fp8 elems per chunk: gr, w
PIPE = 2               # software pipeline depth

LAST_RUN_INFO = {}
_NC_CACHE = {}


def _build_kernel():
    nc = bacc.Bacc(None, target_bir_lowering=False)

    pkb_d = nc.dram_tensor("pkb", [P, NCH * CBB], BF16, kind="ExternalInput")
    pkf_d = nc.dram_tensor("pkf", [P, NCH * CBF], FP8, kind="ExternalInput")
    id_d = nc.dram_tensor("ident", [P, P], FP8, kind="ExternalInput")
    out_d = nc.dram_tensor("out", [P, 21], F32, kind="ExternalOutput")

    Sq = mybir.ActivationFunctionType.Square
    Abs = mybir.ActivationFunctionType.Abs

    with tile.TileContext(nc) as tc:
        with (
            tc.tile_pool(name="statics", bufs=1) as statics,
            tc.tile_pool(name="sbuf", bufs=3) as pool,
            tc.tile_pool(name="psum", bufs=PIPE + 1, space="PSUM") as psum,
        ):
            acc = statics.tile([P, 21], F32)
            ident = statics.tile([P, P], FP8)
            nc.sync.dma_start(out=ident[:], in_=id_d[:])

            st = {}

            def stage_load(ci):
                ob = ci * CBB
                of = ci * CBF
                pkb = pool.tile([P, CBB], BF16)
                nc.sync.dma_start(out=pkb[:], in_=pkb_d[:, ob : ob + CBB])
                pkf = pool.tile([P, CBF], FP8)
                nc.sync.dma_start(out=pkf[:], in_=pkf_d[:, of : of + CBF])
                diff = pool.tile([P, 3 * E], BF16)
                lsub = pool.tile([P, C3], BF16)
                ps = [psum.tile([P, H3], F32, name=f"ps{h}") for h in range(2)]
                st[ci] = (pkb, pkf, diff, lsub, ps)

            def stage_a(ci):
                pkb, pkf, diff, lsub, ps = st[ci]
                # TensorE: s3[half] = sum_k r_j via identity copy-accumulate
                grv = pkf[:, : 3 * E].rearrange("p (k h f) -> p k h f", k=K, h=2)
                for h in range(2):
                    for j in range(K):
                        nc.tensor.matmul(
                            ps[h][:],
                            ident[:],
                            grv[:, j, h, :],
                            start=(j == 0),
                            stop=(j == K - 1),
                        )
                # DVE: diff = p_j - p_i with stride-0 middle-dim broadcast
                gp_v = pkb[:, : 3 * E].rearrange("p (d k c) -> p d k c", d=3, k=K)
                pc_b = (
                    pkb[:, 4 * E : 4 * E + C3]
                    .rearrange("p (d c) -> p d c", d=3)
                    .unsqueeze(2)
                    .broadcast_to([P, 3, K, CHUNK])
                )
                diff_v = diff[:].rearrange("p (d k c) -> p d k c", d=3, k=K)
                nc.vector.tensor_sub(diff_v, gp_v, pc_b)
                # ACT: squares in place
                nc.scalar.activation(diff[:], diff[:], Sq)

            def stage_b(ci):
                pkb, pkf, diff, lsub, ps = st[ci]
                dist_v = pkb[:, 3 * E : 4 * E]
                w_v = pkf[:, 3 * E :]
                # u = sq_x - dist + sq_y + sq_z (in place in the x-plane)
                nc.vector.tensor_sub(diff[:, :E], diff[:, :E], dist_v)
                nc.vector.tensor_add(diff[:, :E], diff[:, :E], diff[:, E : 2 * E])
                nc.vector.tensor_add(diff[:, :E], diff[:, :E], diff[:, 2 * E : 3 * E])
                # t = u * w (GpSimd) into the y-plane; |t| summed on ACT
                nc.gpsimd.tensor_mul(diff[:, E : 2 * E], diff[:, :E], w_v)
                nc.scalar.activation(
                    diff[:, 2 * E : 3 * E],
                    diff[:, E : 2 * E],
                    Abs,
                    accum_out=acc[:, ci : ci + 1],
                )
                # LDA: l = s3 - K*(p_i - q_i) per half; |l| summed on ACT
                pq0 = 4 * E + C3
                for h in range(2):
                    lz = lsub[:, h * H3 : (h + 1) * H3]
                    nc.vector.tensor_sub(
                        lz, ps[h][:], pkb[:, pq0 + h * H3 : pq0 + (h + 1) * H3]
                    )
                    nc.scalar.activation(
                        lz, lz, Abs, accum_out=acc[:, 7 + 2 * ci + h : 8 + 2 * ci + h]
                    )
                del st[ci]

            for ci in range(NCH + PIPE):
                if ci < NCH:
                    stage_load(ci)
                    stage_a(ci)
                if ci >= PIPE:
                    stage_b(ci - PIPE)

            nc.sync.dma_start(out=out_d[:], in_=acc[:])

    nc.compile()
    return nc


def _get_nc():
    key = (ROWS, CHUNK)
    if key not in _NC_CACHE:
        _NC_CACHE[key] = _build_kernel()
    return _NC_CACHE[key]


def _shard_inputs(pc_tr, init_pos, idx_any, dists, weights):
    R = P * ROWS
    base = N // N_CORES
    f8 = ml_dtypes.float8_e4m3
    bf = ml_dtypes.bfloat16

    pc = np.ascontiguousarray(np.asarray(pc_tr, dtype=np.float32))
    q = np.ascontiguousarray(np.asarray(init_pos, dtype=np.float32))
    idx = np.asarray(idx_any, dtype=np.int64)
    dist = np.asarray(dists, dtype=np.float32)
    w = np.asarray(weights, dtype=np.float32)

    r_tab = pc - q
    ident = np.eye(P, dtype=np.float32)

    in_maps = []
    for c in range(N_CORES):
        sl = slice(c * base, (c + 1) * base)
        idx_c = idx[sl].ravel()

        # gathered neighbor positions -> (d, k, c) per chunk
        gp_e = np.empty((R, K, 3), np.float32)
        np.take(pc, idx_c, axis=0, out=gp_e[:base].reshape(-1, 3))
        gp_e[base:] = pc[0]
        gp_s = gp_e.reshape(P, NCH, CHUNK, K, 3).transpose(0, 1, 4, 3, 2)

        dist_s = np.zeros((R, K), np.float32)
        dist_s[:base] = dist[sl]
        dist_kc = dist_s.reshape(P, NCH, CHUNK, K).transpose(0, 1, 3, 2)
        w_s = np.zeros((R, K), np.float32)
        w_s[:base] = w[sl]
        w_kc = w_s.reshape(P, NCH, CHUNK, K).transpose(0, 1, 3, 2)

        pc_e = np.empty((R, 3), np.float32)
        pc_e[:base] = pc[sl]
        pc_e[base:] = pc[0]
        pc_s = pc_e.reshape(P, NCH, CHUNK, 3).transpose(0, 1, 3, 2)

        pq_e = np.empty((R, 3), np.float32)
        pq_e[:base] = pc[sl] - q[sl]
        pq_e[base:] = r_tab[0]
        pqk_s = (K * pq_e).reshape(P, NCH, 2, H3)

        pkb = np.empty((P, NCH, CBB), bf)
        pkb[:, :, : 3 * E] = gp_s.reshape(P, NCH, 3 * E).astype(bf)
        pkb[:, :, 3 * E : 4 * E] = dist_kc.reshape(P, NCH, E).astype(bf)
        pkb[:, :, 4 * E : 4 * E + C3] = pc_s.reshape(P, NCH, C3).astype(bf)
        pkb[:, :, 4 * E + C3 :] = pqk_s.reshape(P, NCH, C3).astype(bf)

        # gathered r -> (k, h, c, d) per chunk for TensorE accumulation
        gr_e = np.empty((R, K, 3), np.float32)
        np.take(r_tab, idx_c, axis=0, out=gr_e[:base].reshape(-1, 3))
        gr_e[base:] = r_tab[0]
        gr_s = gr_e.reshape(P, NCH, 2, HC, K, 3).transpose(0, 1, 4, 2, 3, 5)

        pkf = np.empty((P, NCH, CBF), f8)
        pkf[:, :, : 3 * E] = gr_s.reshape(P, NCH, 3 * E).astype(f8)
        pkf[:, :, 3 * E :] = w_kc.reshape(P, NCH, E).astype(f8)

        in_maps.append(
            {
                "pkb": pkb.reshape(P, NCH * CBB),
                "pkf": pkf.reshape(P, NCH * CBF),
                "ident": ident.astype(f8),
            }
        )
    return in_maps


def kernel(pc_transformed, nn_init_positions, nn_indices, nn_distances, neighbor_weights):
    nc = _get_nc()
    in_maps = _shard_inputs(
        pc_transformed, nn_init_positions, nn_indices, nn_distances, neighbor_weights
    )
    try:
        res = run_bass_kernel_spmd(
            nc, in_maps, core_ids=list(range(N_CORES)), trace=True
        )
    except Exception:
        res = run_bass_kernel_spmd(
            nc, in_maps, core_ids=list(range(N_CORES)), trace=False
        )
    LAST_RUN_INFO["exec_time_ns"] = res.exec_time_ns
    LAST_RUN_INFO["mean_exec_time_ns"] = res.mean_exec_time_ns

    t1 = sum(
        float(res.results[i]["out"][:, :7].astype(np.float64).sum())
        for i in range(N_CORES)
    )
    t2 = sum(
        float(res.results[i]["out"][:, 7:21].astype(np.float64).sum())
        for i in range(N_CORES)
    )
    loss = t1 / (N * K) + LDA_WEIGHT * (t2 / K) / (N * 3)
    return np.float32(loss)
